# revision 44
# baseline (speedup 1.0000x reference)
"""LAHGCN hypergraph-conv kernel for 8 Trainium2 NeuronCores.

Math (per reference):
  smooth(x) = Dv^-1/2 H De^-1 H^T Dv^-1/2 x  (S),  branches k=0..3:
  hidden_k = relu(S(x_k W1_k + 1 b1_k));  out = concat(hidden) W2 + b2;  res = S out.

Design:
  Phase A (node-sharded): y = dv*(xW1 + 1b1), bf16, x SBUF-resident.
    y AllGathered in 4 row-chunks (overlaps A's tail / B's lo prefix).
  Phase B (edge-sharded): ef = de*(H^T y) via dma_gather of bf16 rows +
    prebaked fp8 scatter matmuls (per-block row dedup, host-baked index
    streams and multiplicity matrices; no on-chip one-hot construction).
    Blocks reordered with a lo-stream prefix to hide the y AllGather;
    ef AllGathered in 4 chunks emitted mid-loop.
  Phase C (node-sharded): u = relu(H ef) via gathers; y2 = dv^2*(u@W2).
  Phase D/E (64-ch smooth): dense incidence-matrix matmuls out_T[ch,e] =
    sum_n y2[n,ch] H[n,e] with H staged in fp8 (0/1/2 exact), streamed
    sequentially -- no per-row gather descriptors at all.
  b2 via host-side rank-1 s1 = S@1 correction.
"""
import numpy as np

N, E, NNZ = 50000, 20000, 1600000
CONCAT, C_IN, C_HID = 4, 256, 256
C = CONCAT * C_HID            # 1024
C_OUT, C_OUT_P = 40, 64
W = 8
NPC_R, EPC_R = N // W, E // W           # 6250, 2500 real per core
NBLK, EBLK = 49, 20
NPC, EPC = NBLK * 128, EBLK * 128       # 6272, 2560 padded per core
NP_, EP_ = W * NPC, W * EPC             # 50176, 20480
NPCH = NPC // 2                         # 3136 (per-core y half)
NHALF = W * NPCH                        # 25088 rows in each y_full half
NQ = NPC // 4                           # 1568 (per-core y AG quarter)
EQ = EPC // 4                           # 640  (per-core ef AG quarter)
PRE_B = 4                               # lo-stream prefix blocks in phase B
BATCH = 8                               # gather chunks per dma_gather (64 desc/engine packet cap)
NB_G, EB_G = W * NBLK, W * EBLK         # 392, 160 global blocks
ECH = EPC // 512                        # 5 edge chunks of 512 (phase D psums)
NCH = 13                                # node chunks of 512 (phase E): 6656 rows
NOUT_P = NCH * 512                      # 6656
NCH1, NCH2 = 7, 6                       # phase E psum passes
NBLK_PAD = NCH * 4                      # 52 dv columns (E writes 6656 rows)


def _wrap_idx(idx):
    """[L] int -> [128, L/16] int16 wrapped layout, replicated across q7 cores."""
    L = len(idx)
    assert L % 16 == 0
    a = np.full((16, L // 16), 0, np.int16)
    a[np.arange(L) % 16, np.arange(L) // 16] = idx.astype(np.int16)
    return np.tile(a, (8, 1))


def _streams(rows, segpos, nblk, K, f8):
    """Dedup per block; build flat index stream [nblk*K*128] of DISTINCT rows
    plus a prebaked scatter matrix oh [128, nblk*K*128] fp8 where
    oh[slot%128, (b*K + slot//128)*128 + pos] = multiplicity."""
    L = nblk * K * 128
    idx = np.zeros(L, np.int64)
    oh = np.zeros(128 * L, np.uint8)
    for b in range(nblk):
        r, p = rows[b], segpos[b]
        u, inv = np.unique(r, return_inverse=True)
        n = len(u)
        assert n <= K * 128
        base = b * K * 128
        idx[base:base + n] = u
        slot = inv
        col = (b * K + slot // 128) * 128 + p.astype(np.int64)
        np.add.at(oh, (slot % 128) * L + col, 1)
    return idx, oh.reshape(128, L).astype(f8)


def _prep(node_idx, edge_idx, dv_is, de_inv, f8):
    """Host-side index prep for gather phases B/C. Per-core dict of arrays.

    y is AllGathered in four per-core quarters (two per lo/hi half tensor);
    ef in four quarters into row slices of the single ef_full."""
    nloc = node_idx % NPC_R
    ncore = node_idx // NPC_R
    q2 = nloc // NQ                       # 0..3
    islo = q2 < 2
    yrow = (q2 % 2) * (W * NQ) + ncore * NQ + nloc % NQ
    el = edge_idx % EPC_R
    ecore = edge_idx // EPC_R
    erow = (el // EQ) * (W * EQ) + ecore * EQ + el % EQ    # edge -> ef row
    # dir1: sort by edge
    p1 = np.argsort(edge_idx, kind="stable")
    e1, y1, l1 = edge_idx[p1], yrow[p1], islo[p1]
    # dir2: sort by node
    p2 = np.argsort(node_idx, kind="stable")
    n2, e2 = node_idx[p2], erow[p2]
    per = []
    for c in range(W):
        m1 = (e1 >= c * EPC_R) & (e1 < (c + 1) * EPC_R)
        el = e1[m1] - c * EPC_R
        yr = y1[m1]
        lo = l1[m1]
        lo_rows, lo_pos, hi_rows, hi_pos = [], [], [], []
        for b in range(EBLK):
            mb = (el >= b * 128) & (el < (b + 1) * 128)
            rb, pb, lb = yr[mb], el[mb] - b * 128, lo[mb]
            lo_rows.append(rb[lb]); lo_pos.append(pb[lb])
            hi_rows.append(rb[~lb]); hi_pos.append(pb[~lb])
        m2 = (n2 >= c * NPC_R) & (n2 < (c + 1) * NPC_R)
        nl = n2[m2] - c * NPC_R
        er = e2[m2]
        c_rows, c_pos = [], []
        for b in range(NBLK):
            mb = (nl >= b * 128) & (nl < (b + 1) * 128)
            c_rows.append(er[mb]); c_pos.append(nl[mb] - b * 128)
        per.append((lo_rows, lo_pos, hi_rows, hi_pos, c_rows, c_pos))
    nuniq = lambda r: len(np.unique(r))
    KA = max(max((nuniq(r) + 127) // 128 for r in p[0]) for p in per)
    KB = max(max((nuniq(r) + 127) // 128 for r in p[2]) for p in per)
    KC = max(max((nuniq(r) + 127) // 128 for r in p[4]) for p in per)
    KA, KB, KC = max(KA, 1), max(KB, 1), max(KC, 1)
    cores = []
    for c in range(W):
        lo_rows, lo_pos, hi_rows, hi_pos, c_rows, c_pos = per[c]
        iA, sA = _streams(lo_rows, lo_pos, EBLK, KA, f8)
        iB, sB = _streams(hi_rows, hi_pos, EBLK, KB, f8)
        iC, sC = _streams(c_rows, c_pos, NBLK, KC, f8)
        dv = np.zeros(NPC, np.float32)
        dv[:NPC_R] = dv_is[c * NPC_R:(c + 1) * NPC_R]
        de = np.zeros(EPC, np.float32)
        de[:EPC_R] = de_inv[c * EPC_R:(c + 1) * EPC_R]
        dvp = np.zeros((128, NBLK_PAD), np.float32)
        dvp[:, :NBLK] = dv.reshape(NBLK, 128).T
        cores.append(dict(
            idxA=_wrap_idx(iA), ohA=sA,
            idxB=_wrap_idx(iB), ohB=sB,
            idxC=_wrap_idx(iC), ohC=sC,
            dv_blk=dvp,
            dvsq_blk=(dv * dv).reshape(NBLK, 128).T.copy(),
            de_blk=de.reshape(EBLK, 128).T.copy()))
    return cores, KA, KB, KC


def _dense_h(node_idx, edge_idx, f8):
    """Dense incidence matrices for phases D/E, per core, fp8.

    H_D[c]: [392 nb, 128 p, 2560 e]  (node-blocks x local edge cols)
    H_E1/H_E2[c]: [160 eb, 128 p, 7*512 / 6*512]  (edge-blocks x local node cols)
    """
    nrow = (node_idx // NPC_R) * NPC + node_idx % NPC_R
    erow = (edge_idx // EPC_R) * EPC + edge_idx % EPC_R
    ecore = edge_idx // EPC_R
    ncore = node_idx // NPC_R
    el = edge_idx % EPC_R
    nl = node_idx % NPC_R
    HD, HE1, HE2 = [], [], []
    for c in range(W):
        m = ecore == c
        nb = (nrow[m] // 128).astype(np.int64)
        p = nrow[m] % 128
        e = el[m].astype(np.int64)
        hd = np.zeros(NB_G * 128 * EPC, np.uint8)
        np.add.at(hd, (nb * 128 + p) * EPC + e, 1)
        HD.append(hd.reshape(NB_G, 128, EPC).astype(f8))

        m = ncore == c
        eb = (erow[m] // 128).astype(np.int64)
        p = erow[m] % 128
        n = nl[m].astype(np.int64)
        h1m = n < NCH1 * 512
        he1 = np.zeros(EB_G * 128 * (NCH1 * 512), np.uint8)
        np.add.at(he1, (eb[h1m] * 128 + p[h1m]) * (NCH1 * 512) + n[h1m], 1)
        HE1.append(he1.reshape(EB_G, 128, NCH1 * 512).astype(f8))
        he2 = np.zeros(EB_G * 128 * (NCH2 * 512), np.uint8)
        np.add.at(he2, (eb[~h1m] * 128 + p[~h1m]) * (NCH2 * 512)
                  + (n[~h1m] - NCH1 * 512), 1)
        HE2.append(he2.reshape(EB_G, 128, NCH2 * 512).astype(f8))
    return HD, HE1, HE2


def _build(KA, KB, KC):
    import concourse.bass as bass
    import concourse.mybir as mybir
    from concourse import bacc, masks
    from concourse.tile import TileContext

    f32, i16 = mybir.dt.float32, mybir.dt.int16
    bf16, f8 = mybir.dt.bfloat16, mybir.dt.float8e4
    nc = bacc.Bacc("TRN2", num_devices=W)
    T = lambda n, s, d: nc.dram_tensor(n, s, d, kind="ExternalInput")
    xT = T("xT", [CONCAT, C_IN, NPC], bf16)
    W1 = T("W1", [CONCAT, C_IN, C_HID], bf16)
    b1c = T("b1c", [1, C], bf16)
    W2p = T("W2p", [C, C_OUT_P], bf16)
    dv_blk = T("dv_blk", [128, NBLK_PAD], f32); dvsq_blk = T("dvsq_blk", [128, NBLK], f32)
    de_blk = T("de_blk", [128, EBLK], f32)
    idxA = T("idxA", [128, EBLK * KA * 8], i16); ohA = T("ohA", [128, EBLK * KA * 128], f8)
    idxB = T("idxB", [128, EBLK * KB * 8], i16); ohB = T("ohB", [128, EBLK * KB * 128], f8)
    idxC = T("idxC", [128, NBLK * KC * 8], i16); ohC = T("ohC", [128, NBLK * KC * 128], f8)
    HD = T("HD", [NB_G, 128, EPC], f8)
    HE1 = T("HE1", [EB_G, 128, NCH1 * 512], f8)
    HE2 = T("HE2", [EB_G, 128, NCH2 * 512], f8)
    out_own = nc.dram_tensor("out_own", [NOUT_P, C_OUT_P], f32, kind="ExternalOutput")
    I = lambda n, s, d: nc.dram_tensor(n, s, d, kind="Internal")
    S = lambda n, s, d: nc.dram_tensor(n, s, d, kind="Internal", addr_space="Shared")
    y_own = I("y_own", [NPC, C], bf16)
    y_full_lo = S("y_full_lo", [NHALF, C], bf16)
    y_full_hi = S("y_full_hi", [NHALF, C], bf16)
    ef_own, ef_full = I("ef_own", [EPC, C], bf16), S("ef_full", [EP_, C], bf16)
    y2_own = I("y2_own", [128, NBLK * C_OUT_P], bf16)
    y2_full = S("y2_full", [W * 128, NBLK * C_OUT_P], bf16)
    ef2_own = I("ef2_own", [128, EBLK * C_OUT_P], bf16)
    ef2_full = S("ef2_full", [W * 128, EBLK * C_OUT_P], bf16)
    RG = [list(range(W))]
    AG = lambda i_ap, o_ap: nc.gpsimd.collective_compute(
        "AllGather", mybir.AluOpType.bypass, replica_groups=RG, ins=[i_ap], outs=[o_ap])

    with TileContext(nc) as tc:
        with tc.tile_pool(name="const", bufs=1) as cp:
            w1_sb = cp.tile([128, CONCAT * 2 * C_HID], bf16)     # [k][q] -> 256 cols
            for k in range(CONCAT):
                for q in range(2):
                    nc.sync.dma_start(
                        w1_sb[:, (k * 2 + q) * C_HID:(k * 2 + q + 1) * C_HID],
                        W1[k, q * 128:(q + 1) * 128, :])
            w2_sb = cp.tile([128, 8 * C_OUT_P], bf16)
            for f in range(8):
                nc.sync.dma_start(w2_sb[:, f * C_OUT_P:(f + 1) * C_OUT_P],
                                  W2p[f * 128:(f + 1) * 128, :])
            b1_sb = cp.tile([1, C], bf16); nc.sync.dma_start(b1_sb[:], b1c[:])
            ones_sb = cp.tile([1, 128], bf16); nc.vector.memset(ones_sb[:], 1.0)
            ident = cp.tile([128, 128], bf16); masks.make_identity(nc, ident[:])
            ident32 = cp.tile([128, 128], f32); masks.make_identity(nc, ident32[:])
            dv_sb = cp.tile([128, NBLK_PAD], f32); nc.sync.dma_start(dv_sb[:], dv_blk[:])
            dvsq_sb = cp.tile([128, NBLK], f32); nc.sync.dma_start(dvsq_sb[:], dvsq_blk[:])
            de_sb = cp.tile([128, EBLK], f32); nc.sync.dma_start(de_sb[:], de_blk[:])
            iA = cp.tile([128, EBLK * KA * 8], i16); nc.sync.dma_start(iA[:], idxA[:])
            iB = cp.tile([128, EBLK * KB * 8], i16); nc.sync.dma_start(iB[:], idxB[:])
            iC = cp.tile([128, NBLK * KC * 8], i16); nc.sync.dma_start(iC[:], idxC[:])

            mm = lambda *a, **kw: nc.tensor.matmul(*a, skip_group_check=True, **kw)

            def seg_pass(K, idx_sb, oh_d, src_ap, elem, pool, ps,
                         start_stream, stop_stream, tag):
                """Gather + prebaked-scatter-matmul accumulation over one block."""
                oh = pool.tile([128, K * 128], f8, tag="oh" + tag)
                nc.sync.dma_start(oh[:], oh_d)
                nbat = (K + BATCH - 1) // BATCH
                for s in range(nbat):
                    k0 = s * BATCH
                    nch = min(BATCH, K - k0)
                    g = pool.tile([128, BATCH, elem], bf16, tag="gat")
                    nc.gpsimd.dma_gather(
                        out_ap=g[:, :nch, :], in_ap=src_ap,
                        idxs_ap=idx_sb[:, k0 * 8:(k0 + nch) * 8],
                        num_idxs=nch * 128, num_idxs_reg=nch * 128,
                        elem_size=elem)
                    for j in range(nch):
                        first = start_stream and (s == 0 and j == 0)
                        last = stop_stream and (k0 + j == K - 1)
                        for h in range((elem + 511) // 512):
                            w_ = min(512, elem - h * 512)
                            mm(ps[:, h * 512:h * 512 + w_],
                               lhsT=oh[:, (k0 + j) * 128:(k0 + j + 1) * 128],
                               rhs=g[:, j, h * 512:h * 512 + w_],
                               start=first, stop=last)

            # ---- phase A: y = dv * (x @ W1 + 1 b1) ----
            with tc.tile_pool(name="pax", bufs=1) as pax, \
                 tc.tile_pool(name="pa", bufs=4) as pa, \
                 tc.tile_pool(name="pap", bufs=3, space="PSUM") as pap:
                x_sb = pax.tile([128, CONCAT * 2 * NPC], bf16)
                for k in range(CONCAT):
                    for q in range(2):
                        nc.sync.dma_start(
                            x_sb[:, (k * 2 + q) * NPC:(k * 2 + q + 1) * NPC],
                            xT[k, q * 128:(q + 1) * 128, :])
                for b in range(NBLK):
                    ps = pap.tile([128, C], f32, tag="psA")
                    mm(ps[:, :512], lhsT=ones_sb[:, :], rhs=b1_sb[:, :512], start=True, stop=False)
                    mm(ps[:, 512:], lhsT=ones_sb[:, :], rhs=b1_sb[:, 512:], start=True, stop=False)
                    for k in range(CONCAT):
                        for q in range(2):
                            mm(ps[:, k * C_HID:(k + 1) * C_HID],
                               lhsT=x_sb[:, (k * 2 + q) * NPC + b * 128:
                                         (k * 2 + q) * NPC + (b + 1) * 128],
                               rhs=w1_sb[:, (k * 2 + q) * C_HID:(k * 2 + q + 1) * C_HID],
                               start=False, stop=(q == 1))
                    y_sb = pa.tile([128, C], bf16, tag="ysb")
                    nc.vector.tensor_tensor(
                        out=y_sb[:], in0=ps[:],
                        in1=dv_sb[:, b:b + 1].broadcast_to([128, C]),
                        op=mybir.AluOpType.mult)
                    nc.sync.dma_start(y_own[b * 128:(b + 1) * 128, :], y_sb[:])
            for q in range(4):
                yf = y_full_lo if q < 2 else y_full_hi
                AG(y_own[q * NQ:(q + 1) * NQ, :],
                   yf[(q % 2) * W * NQ:(q % 2 + 1) * W * NQ, :])

            # ---- phase B: ef = de * (H^T y) over own edges ----
            with tc.tile_pool(name="pb", bufs=4) as pb, \
                 tc.tile_pool(name="pbp", bufs=1, space="PSUM") as pbp:
                psB = {}

                def lo_pass(b):
                    psB[b] = pbp.tile([128, C], f32, tag="psB%d" % (b % PRE_B),
                                      name="psB_%d" % b)
                    seg_pass(KA, iA[:, b * KA * 8:(b + 1) * KA * 8],
                             ohA[:, b * KA * 128:(b + 1) * KA * 128], y_full_lo[:],
                             C, pb, psB[b], True, False, "A")

                def hi_tail(b):
                    ps = psB.pop(b)
                    seg_pass(KB, iB[:, b * KB * 8:(b + 1) * KB * 8],
                             ohB[:, b * KB * 128:(b + 1) * KB * 128], y_full_hi[:],
                             C, pb, ps, False, True, "B")
                    ef_sb = pb.tile([128, C], bf16, tag="efsb")
                    nc.vector.tensor_tensor(
                        out=ef_sb[:], in0=ps[:],
                        in1=de_sb[:, b:b + 1].broadcast_to([128, C]),
                        op=mybir.AluOpType.mult)
                    nc.sync.dma_start(ef_own[b * 128:(b + 1) * 128, :], ef_sb[:])

                for b in range(PRE_B):
                    lo_pass(b)
                for b in range(EBLK):
                    hi_tail(b)
                    if b + PRE_B < EBLK:
                        lo_pass(b + PRE_B)
                    if b in (4, 9, 14):
                        q = b // 5
                        AG(ef_own[q * EQ:(q + 1) * EQ, :],
                           ef_full[q * W * EQ:(q + 1) * W * EQ, :])
            AG(ef_own[3 * EQ:4 * EQ, :], ef_full[3 * W * EQ:4 * W * EQ, :])

            # ---- phase C: u = relu(H ef); y2 = dv^2 * (u @ W2) ----
            with tc.tile_pool(name="pcy", bufs=1) as pcy, \
                 tc.tile_pool(name="pc", bufs=4) as pc, \
                 tc.tile_pool(name="pcp", bufs=3, space="PSUM") as pcp, \
                 tc.tile_pool(name="pct", bufs=1, space="PSUM") as pct:
                y2acc = pcy.tile([128, NBLK * C_OUT_P], bf16)
                for b in range(NBLK):
                    pz = pcp.tile([128, C], f32, tag="psC")
                    seg_pass(KC, iC[:, b * KC * 8:(b + 1) * KC * 8],
                             ohC[:, b * KC * 128:(b + 1) * KC * 128], ef_full[:],
                             C, pc, pz, True, True, "C")
                    u_sb = pc.tile([128, C], bf16, tag="usb")
                    nc.scalar.activation(out=u_sb[:], in_=pz[:],
                                         func=mybir.ActivationFunctionType.Relu)
                    pt = pct.tile([128, C], bf16, tag="ptC")
                    for f in range(8):
                        nc.tensor.transpose(pt[:, f * 128:(f + 1) * 128],
                                            u_sb[:, f * 128:(f + 1) * 128], ident[:])
                    ut_sb = pc.tile([128, C], bf16, tag="utsb")
                    nc.vector.tensor_copy(ut_sb[:], pt[:])
                    po = pct.tile([128, C_OUT_P], f32, tag="poC")
                    for f in range(8):
                        mm(po[:], lhsT=ut_sb[:, f * 128:(f + 1) * 128],
                           rhs=w2_sb[:, f * C_OUT_P:(f + 1) * C_OUT_P],
                           start=(f == 0), stop=(f == 7))
                    nc.vector.tensor_tensor(
                        out=y2acc[:, b * C_OUT_P:(b + 1) * C_OUT_P], in0=po[:],
                        in1=dvsq_sb[:, b:b + 1].broadcast_to([128, C_OUT_P]),
                        op=mybir.AluOpType.mult)
                nc.sync.dma_start(y2_own[:], y2acc[:])
            AG(y2_own[:], y2_full[:])

            # ---- phases D/E: dense fp8 H, transposed psums; shared SBUF pools
            # so E's H stream prefetches during D ----
            with tc.tile_pool(name="pdy", bufs=1) as pdy, \
                 tc.tile_pool(name="pdh", bufs=12) as pdh, \
                 tc.tile_pool(name="peh", bufs=8) as peh, \
                 tc.tile_pool(name="pd", bufs=3) as pd:
              with tc.tile_pool(name="pdp", bufs=1, space="PSUM") as pdp, \
                   tc.tile_pool(name="pdt", bufs=2, space="PSUM") as pdt:
                y2sb = pdy.tile([128, W * NBLK * C_OUT_P], bf16)
                for c2 in range(W):
                    nc.sync.dma_start(
                        y2sb[:, c2 * NBLK * C_OUT_P:(c2 + 1) * NBLK * C_OUT_P],
                        y2_full[c2 * 128:(c2 + 1) * 128, :])
                ef2acc = pdy.tile([128, EBLK * C_OUT_P], bf16)
                psD = [pdp.tile([64, 512], f32, tag="psD%d" % e, name="psD%d" % e)
                       for e in range(ECH)]
                for nb in range(NB_G):
                    h_sb = pdh.tile([128, EPC], f8, tag="hD")
                    nc.sync.dma_start(h_sb[:], HD[nb])
                    for e in range(ECH):
                        mm(psD[e][:, :], lhsT=y2sb[:, nb * C_OUT_P:(nb + 1) * C_OUT_P],
                           rhs=h_sb[:, e * 512:(e + 1) * 512],
                           start=(nb == 0), stop=(nb == NB_G - 1))
                for e in range(ECH):
                    e2 = pd.tile([64, 512], bf16, tag="e2")
                    nc.vector.tensor_copy(e2[:], psD[e][:, :])
                    for j in range(4):
                        ptr = pdt.tile([128, C_OUT_P], bf16, tag="ptr")
                        nc.tensor.transpose(ptr[:, :], e2[:, j * 128:(j + 1) * 128],
                                            ident[:64, :64])
                        eb = e * 4 + j
                        nc.vector.tensor_tensor(
                            out=ef2acc[:, eb * C_OUT_P:(eb + 1) * C_OUT_P],
                            in0=ptr[:, :],
                            in1=de_sb[:, eb:eb + 1].broadcast_to([128, C_OUT_P]),
                            op=mybir.AluOpType.mult)
                nc.sync.dma_start(ef2_own[:], ef2acc[:])
              AG(ef2_own[:], ef2_full[:])

              # ---- phase E: res = dv * (H ef2) ----
              with tc.tile_pool(name="pep", bufs=1, space="PSUM") as pep, \
                   tc.tile_pool(name="pet", bufs=1, space="PSUM") as pet:
                efsb = pdy.tile([128, W * EBLK * C_OUT_P], bf16, tag="efsb2")
                for c2 in range(W):
                    nc.sync.dma_start(
                        efsb[:, c2 * EBLK * C_OUT_P:(c2 + 1) * EBLK * C_OUT_P],
                        ef2_full[c2 * 128:(c2 + 1) * 128, :])
                pe_ = pd
                for pss, (HE, nchs, nch0) in enumerate(
                        [(HE1, NCH1, 0), (HE2, NCH2, NCH1)]):
                    psE = [pep.tile([64, 512], f32, tag="psE%d" % t,
                                    name="psE%d_%d" % (pss, t))
                           for t in range(nchs)]
                    for eb in range(EB_G):
                        h_sb = peh.tile([128, NCH1 * 512], f8, tag="hE")
                        nc.sync.dma_start(h_sb[:, :nchs * 512], HE[eb])
                        for t in range(nchs):
                            mm(psE[t][:, :],
                               lhsT=efsb[:, eb * C_OUT_P:(eb + 1) * C_OUT_P],
                               rhs=h_sb[:, t * 512:(t + 1) * 512],
                               start=(eb == 0), stop=(eb == EB_G - 1))
                    for t in range(nchs):
                        nch = nch0 + t
                        osb = pe_.tile([64, 512], f32, tag="osb")
                        nc.vector.tensor_copy(osb[:], psE[t][:, :])
                        for j in range(4):
                            pto = pet.tile([128, C_OUT_P], f32, tag="pto")
                            nc.tensor.transpose(pto[:, :], osb[:, j * 128:(j + 1) * 128],
                                                ident32[:64, :64])
                            b_ = nch * 4 + j
                            ob = pe_.tile([128, C_OUT_P], f32, tag="ob")
                            nc.vector.tensor_tensor(
                                out=ob[:, :], in0=pto[:, :],
                                in1=dv_sb[:, b_:b_ + 1].broadcast_to([128, C_OUT_P]),
                                op=mybir.AluOpType.mult)
                            nc.sync.dma_start(
                                out_own[nch * 512 + j * 128:nch * 512 + (j + 1) * 128, :],
                                ob[:, :])
    nc.finalize()
    return nc


_CACHE = {}


def kernel(x_list, W1, b1, W2, b2, node_idx, edge_idx, n_edges, _trace=False):
    from concourse import bass_utils
    import concourse.mybir as mybir
    bf16 = mybir.dt.np(mybir.dt.bfloat16)
    f8 = mybir.dt.np(mybir.dt.float8e4)
    x_list = np.asarray(x_list, np.float32); W1 = np.asarray(W1, np.float32)
    b1 = np.asarray(b1, np.float32); W2 = np.asarray(W2, np.float32)
    b2 = np.asarray(b2, np.float32)
    node_idx = np.asarray(node_idx, np.int32); edge_idx = np.asarray(edge_idx, np.int32)

    dv = np.bincount(node_idx, minlength=N).astype(np.float32)
    de = np.bincount(edge_idx, minlength=E).astype(np.float32)
    dv_is = np.where(dv > 0, 1.0 / np.sqrt(np.maximum(dv, 1.0)), 0.0).astype(np.float32)
    de_inv = np.where(de > 0, 1.0 / np.maximum(de, 1.0), 0.0).astype(np.float32)
    # s1 = S @ 1 for the host-side b2 rank-1 term
    ef_t = np.bincount(edge_idx, weights=dv_is[node_idx], minlength=E) * de_inv
    s1 = dv_is * np.bincount(node_idx, weights=ef_t[edge_idx], minlength=N)

    cores, KA, KB, KC = _prep(node_idx, edge_idx, dv_is, de_inv, f8)
    HD, HE1, HE2 = _dense_h(node_idx, edge_idx, f8)
    key = (KA, KB, KC)
    if key not in _CACHE:
        _CACHE[key] = _build(KA, KB, KC)
    nc = _CACHE[key]

    W2p = np.zeros((C, C_OUT_P), np.float32)
    W2p[:, :C_OUT] = W2
    in_maps = []
    for c in range(W):
        xTc = np.zeros((CONCAT, C_IN, NPC), bf16)
        xTc[:, :, :NPC_R] = x_list[:, c * NPC_R:(c + 1) * NPC_R, :].transpose(0, 2, 1).astype(bf16)
        m = dict(xT=xTc, W1=W1.astype(bf16), b1c=b1.reshape(1, C).astype(bf16),
                 W2p=W2p.astype(bf16),
                 HD=HD[c], HE1=HE1[c], HE2=HE2[c], **cores[c])
        in_maps.append(m)
    try:
        res = bass_utils.run_bass_kernel_spmd(nc, in_maps, core_ids=list(range(W)),
                                              trace=_trace)
    except ModuleNotFoundError:
        res = bass_utils.run_bass_kernel_spmd(nc, in_maps, core_ids=list(range(W)),
                                              trace=False)
    out = np.empty((N, C_OUT), np.float32)
    for c in range(W):
        out[c * NPC_R:(c + 1) * NPC_R] = res.results[c]["out_own"][:NPC_R, :C_OUT]
    out += np.outer(s1, b2)
    kernel._last = res
    return out
